# revision 2
# baseline (speedup 1.0000x reference)
"""Sliding-window attention (BERT-style, window +/-256, RoPE) on 8 TRN2 NeuronCores.

Sharding: core c -> batch b = c//4, head-group g = c%4 (4 of 16 heads each).
Per core: Q/K/V projections in fp16 (scores pre-scaled by folding 8.0 = sqrt(HD)
into Wq on host), RoPE via DMA partition-rotation + DVE/GPSIMD muls, banded
scores with the mask added via an identity-matmul into PSUM, full row-max on
DVE, exp on ACT (bias = -rowmax), P/V in bf16, P transposed on the PE,
PV with an appended ones-column for the softmax denominator.

Self-contained: hardcodes shapes; host side only reshapes/casts/concats.
"""
import os
import sys

sys.path.insert(0, "/opt/trn_rl_repo")

import numpy as np
import ml_dtypes

import concourse.bass as bass
import concourse.mybir as mybir
import concourse.tile as tile
from concourse import bacc
from concourse.bass_utils import run_bass_kernel_spmd

F16 = mybir.dt.float16
BF16 = mybir.dt.bfloat16
F32 = mybir.dt.float32
AF = mybir.ActivationFunctionType
ALU = mybir.AluOpType

B, S, D, H, HD = 2, 2048, 1024, 16, 64
WIN = 256
NSTRIP = 640          # key-strip width per 128-query block
NQB = S // 128        # 16 query blocks
HPC = 4               # heads per core
HDPC = HPC * HD       # 256 output dims per core
ROPE_THETA = 10000.0
MASK_VAL = -60000.0   # fp16-exact large negative, added to scaled scores

LAST_EXEC_NS = None
LAST_RESULTS = None


def strip_start(qb):
    return min(max(qb * 128 - WIN, 0), S - NSTRIP)


def mask_info():
    """Per qb: list of (c0, c1) column segments (split at 512) that contain
    any out-of-band cell, plus the packed [128, total] fp16 mask tensor and
    per-(qb,seg) column offsets into it."""
    segs_per_qb = []
    cols = []
    offsets = []
    total = 0
    for qb in range(NQB):
        i0 = qb * 128
        s0 = strip_start(qb)
        ql = np.arange(i0, i0 + 128)[:, None]
        kk = np.arange(s0, s0 + NSTRIP)[None, :]
        valid = (kk >= ql - WIN) & (kk <= ql + WIN)
        bad_col = (~valid).any(axis=0)
        runs = []
        c = 0
        while c < NSTRIP:
            if bad_col[c]:
                c1 = c
                while c1 < NSTRIP and bad_col[c1]:
                    c1 += 1
                if c < 512 < c1:
                    runs.append((c, 512))
                    runs.append((512, c1))
                else:
                    runs.append((c, c1))
                c = c1
            else:
                c += 1
        seg_list = []
        for (c0, c1) in runs:
            m = np.where(valid[:, c0:c1], np.float32(0.0), np.float32(MASK_VAL))
            cols.append(m)
            seg_list.append((c0, c1, total))
            total += c1 - c0
        segs_per_qb.append(seg_list)
    packed = np.concatenate(cols, axis=1).astype(np.float16)
    return segs_per_qb, packed


MASK_SEGS, MASK_PACKED = mask_info()
MASK_COLS = MASK_PACKED.shape[1]


def rope_tables():
    inv_freq = 1.0 / (ROPE_THETA ** (np.arange(0, HD, 2, dtype=np.float32) / HD))
    t = np.arange(S, dtype=np.float32)
    freqs = np.outer(t, inv_freq)                      # [S, 32]
    emb = np.concatenate([freqs, freqs], axis=-1)      # [S, 64]
    cos = np.cos(emb)                                  # [S, 64]
    sin = np.sin(emb)
    # QT layout [hd-part, s]: partition p uses index p % 64; sign of the
    # rotation term folded into the sin table.
    cosT = np.tile(cos.T, (2, 1))                      # [128, S]
    sinT = np.tile(sin.T, (2, 1))
    sign = np.where((np.arange(128) % 64) < 32, -1.0, 1.0)[:, None]
    return cosT.astype(np.float16), (sinT * sign).astype(np.float16)


_NC_CACHE = None


def build():
    nc = bacc.Bacc("TRN2", target_bir_lowering=False, debug=False, num_devices=8)
    xt_d = nc.dram_tensor("xt", [D, S], F16, kind="ExternalInput").ap()
    wq_d = nc.dram_tensor("wq", [D, HDPC], F16, kind="ExternalInput").ap()
    wk_d = nc.dram_tensor("wk", [D, HDPC], F16, kind="ExternalInput").ap()
    wv_d = nc.dram_tensor("wv", [D, HDPC], F16, kind="ExternalInput").ap()
    cos_d = nc.dram_tensor("cosr", [128, S], F16, kind="ExternalInput").ap()
    sin_d = nc.dram_tensor("sinr", [128, S], F16, kind="ExternalInput").ap()
    msk_d = nc.dram_tensor("msk", [128, MASK_COLS], F16, kind="ExternalInput").ap()
    id16_d = nc.dram_tensor("id16", [128, 128], F16, kind="ExternalInput").ap()
    idbf_d = nc.dram_tensor("idbf", [128, 128], BF16, kind="ExternalInput").ap()
    out_d = nc.dram_tensor("out", [S, HDPC], F32, kind="ExternalOutput").ap()

    with tile.TileContext(nc) as tc:
        with (
            tc.tile_pool(name="const", bufs=1) as cpool,
            tc.tile_pool(name="qk", bufs=1) as qkpool,
            tc.tile_pool(name="scratch", bufs=2) as spool,
            tc.tile_pool(name="attn", bufs=3) as apool,
            tc.tile_pool(name="small", bufs=4) as smpool,
            tc.tile_pool(name="ps", bufs=1, space="PSUM") as ps,
        ):
            # ---- loads ----
            xt_sb = cpool.tile([128, 8, S], F16, name="xt_sb")
            nc.sync.dma_start(xt_sb[:], xt_d.rearrange("(kt p) s -> p kt s", p=128))
            w_sb = {}
            for nm, d in (("wq", wq_d), ("wk", wk_d), ("wv", wv_d)):
                t = cpool.tile([128, 8, HDPC], F16, name=nm + "_sb")
                nc.sync.dma_start(t[:], d.rearrange("(kt p) m -> p kt m", p=128))
                w_sb[nm] = t
            cos_sb = cpool.tile([128, S], F16, name="cos_sb")
            nc.sync.dma_start(cos_sb[:], cos_d)
            sin_sb = cpool.tile([128, S], F16, name="sin_sb")
            nc.sync.dma_start(sin_sb[:], sin_d)
            msk_sb = cpool.tile([128, MASK_COLS], F16, name="msk_sb")
            nc.sync.dma_start(msk_sb[:], msk_d)
            id16_sb = cpool.tile([128, 128], F16, name="id16_sb")
            nc.sync.dma_start(id16_sb[:], id16_d)
            idbf_sb = cpool.tile([128, 128], BF16, name="idbf_sb")
            nc.sync.dma_start(idbf_sb[:], idbf_d)

            # ---- V projection: [s-part, hd] blocks with ones column ----
            v_sb = cpool.tile([128, NQB, HPC, HD + 1], BF16, name="v_sb")
            nc.vector.memset(v_sb[:, :, :, HD:HD + 1], 1.0)
            for sb in range(NQB):
                vps = ps.tile([128, HDPC], F32, tag="big", name=f"vps{sb}")
                for kt in range(8):
                    nc.tensor.matmul(vps[:], xt_sb[:, kt, sb * 128:(sb + 1) * 128],
                                     w_sb["wv"][:, kt, :],
                                     start=(kt == 0), stop=(kt == 7))
                nc.scalar.activation(
                    v_sb[:, sb, :, 0:HD],
                    vps[:].rearrange("p (h c) -> p h c", h=HPC),
                    AF.Copy)

            # ---- Q/K projections + RoPE -> [hd-part, s] fp16 ----
            qk_t = {}
            for nm in ("q", "k"):
                for m in range(2):
                    raw = spool.tile([128, S], F16, tag="rope_raw", name=f"{nm}raw{m}")
                    for sc_i in range(4):
                        pps = ps.tile([128, 512], F32, tag="big", name=f"{nm}ps{m}_{sc_i}")
                        for kt in range(8):
                            nc.tensor.matmul(
                                pps[:],
                                w_sb["w" + nm][:, kt, m * 128:(m + 1) * 128],
                                xt_sb[:, kt, sc_i * 512:(sc_i + 1) * 512],
                                start=(kt == 0), stop=(kt == 7))
                        nc.scalar.activation(raw[:, sc_i * 512:(sc_i + 1) * 512],
                                             pps[:], AF.Copy)
                    rot = spool.tile([128, S], F16, tag="rope_rot", name=f"{nm}rot{m}")
                    for gg in range(2):
                        b0 = 64 * gg
                        nc.sync.dma_start(rot[b0:b0 + 32, :], raw[b0 + 32:b0 + 64, :])
                        nc.sync.dma_start(rot[b0 + 32:b0 + 64, :], raw[b0:b0 + 32, :])
                    t1 = spool.tile([128, S], F16, tag="rope_t1", name=f"{nm}t1_{m}")
                    nc.vector.tensor_tensor(out=t1[:], in0=raw[:], in1=cos_sb[:], op=ALU.mult)
                    t2 = spool.tile([128, S], F16, tag="rope_t2", name=f"{nm}t2_{m}")
                    nc.gpsimd.tensor_tensor(out=t2[:], in0=rot[:], in1=sin_sb[:], op=ALU.mult)
                    dst = qkpool.tile([128, S], F16, name=f"{nm}_sb{m}")
                    nc.vector.tensor_tensor(out=dst[:], in0=t1[:], in1=t2[:], op=ALU.add)
                    qk_t[(nm, m)] = dst

            # ---- attention ----
            for h in range(HPC):
                m, hp = h // 2, 64 * (h % 2)
                qs = qk_t[("q", m)]
                ks = qk_t[("k", m)]
                for qb in range(NQB):
                    s0 = strip_start(qb)
                    segs = MASK_SEGS[qb]
                    scp = ps.tile([128, NSTRIP], F32, tag="big", name=f"sc{h}_{qb}")
                    b0_last = max([i for i, (c0, c1, off) in enumerate(segs) if c1 <= 512],
                                  default=None)
                    b1_last = max([i for i, (c0, c1, off) in enumerate(segs) if c0 >= 512],
                                  default=None)
                    nc.tensor.matmul(scp[:, 0:512],
                                     qs[hp:hp + 64, qb * 128:(qb + 1) * 128],
                                     ks[hp:hp + 64, s0:s0 + 512],
                                     start=True, stop=(b0_last is None))
                    nc.tensor.matmul(scp[:, 512:NSTRIP],
                                     qs[hp:hp + 64, qb * 128:(qb + 1) * 128],
                                     ks[hp:hp + 64, s0 + 512:s0 + NSTRIP],
                                     start=True, stop=(b1_last is None))
                    for i, (c0, c1, off) in enumerate(segs):
                        nc.tensor.matmul(scp[:, c0:c1], id16_sb[:],
                                         msk_sb[:, off:off + (c1 - c0)],
                                         start=False,
                                         stop=(i == b0_last or i == b1_last),
                                         skip_group_check=True)
                    negmax = smpool.tile([128, 1], F32, tag="negmax", name=f"nm{h}_{qb}")
                    nc.vector.tensor_reduce(out=negmax[:], in_=scp[:],
                                            axis=mybir.AxisListType.X,
                                            op=ALU.max, negate=True)
                    p_t = apool.tile([128, NSTRIP], BF16, tag="p", name=f"p{h}_{qb}")
                    nc.scalar.activation(p_t[:], scp[:], AF.Exp, bias=negmax[:], scale=1.0)
                    ptp = ps.tile([128, NSTRIP], BF16, tag="ptps", name=f"ptp{h}_{qb}")
                    for j in range(5):
                        nc.tensor.transpose(ptp[:, j * 128:(j + 1) * 128],
                                            p_t[:, j * 128:(j + 1) * 128], idbf_sb[:])
                    pts = apool.tile([128, NSTRIP], BF16, tag="pts", name=f"pts{h}_{qb}")
                    nc.scalar.activation(pts[:], ptp[:], AF.Copy)
                    ctx = ps.tile([128, HD + 1], F32, tag="ctx", name=f"ctx{h}_{qb}")
                    for j in range(5):
                        nc.tensor.matmul(ctx[:], pts[:, j * 128:(j + 1) * 128],
                                         v_sb[:, s0 // 128 + j, h, :],
                                         start=(j == 0), stop=(j == 4))
                    rl = smpool.tile([128, 1], F32, tag="rl", name=f"rl{h}_{qb}")
                    nc.vector.reciprocal(rl[:], ctx[:, HD:HD + 1])
                    o_t = smpool.tile([128, HD], F32, tag="o", name=f"o{h}_{qb}")
                    nc.vector.tensor_scalar(out=o_t[:], in0=ctx[:, 0:HD],
                                            scalar1=rl[:], scalar2=None, op0=ALU.mult)
                    nc.sync.dma_start(
                        out_d[qb * 128:(qb + 1) * 128, h * HD:(h + 1) * HD], o_t[:])
    nc.compile()
    return nc


def kernel(hidden_states, attention_mask, Wq, bq, Wk, bk, Wv, bv):
    global _NC_CACHE, LAST_EXEC_NS, LAST_RESULTS
    hidden_states = np.asarray(hidden_states, dtype=np.float32)
    attention_mask = np.asarray(attention_mask)
    Wq = np.asarray(Wq, dtype=np.float32)
    Wk = np.asarray(Wk, dtype=np.float32)
    Wv = np.asarray(Wv, dtype=np.float32)
    for bias in (bq, bk, bv):
        assert np.all(np.asarray(bias) == 0.0), "nonzero biases unsupported"

    cosT, sinT = rope_tables()
    id16 = np.eye(128, dtype=np.float16)
    idbf = np.eye(128, dtype=np.float32).astype(ml_dtypes.bfloat16)

    xt16 = [np.ascontiguousarray(hidden_states[b].T).astype(np.float16) for b in range(B)]
    in_maps = []
    for c in range(8):
        b, g = c // 4, c % 4
        sl = slice(g * HDPC, (g + 1) * HDPC)
        in_maps.append(dict(
            xt=xt16[b],
            wq=np.ascontiguousarray((Wq[sl, :] * 8.0).T).astype(np.float16),
            wk=np.ascontiguousarray(Wk[sl, :].T).astype(np.float16),
            wv=np.ascontiguousarray(Wv[sl, :].T).astype(np.float16),
            cosr=cosT, sinr=sinT, msk=MASK_PACKED, id16=id16, idbf=idbf,
        ))

    if _NC_CACHE is None:
        _NC_CACHE = build()
    trace = bool(int(os.environ.get("KERNEL_TRACE", "0")))
    res = run_bass_kernel_spmd(_NC_CACHE, in_maps, core_ids=list(range(8)),
                               trace=trace)
    LAST_EXEC_NS = res.exec_time_ns
    LAST_RESULTS = res

    out = np.empty((B, S, D), np.float32)
    for c in range(8):
        b, g = c // 4, c % 4
        out[b, :, g * HDPC:(g + 1) * HDPC] = res.results[c]["out"]
    qmask = (np.asarray(attention_mask) > 0).astype(np.float32)[:, :, None]
    return out * qmask


def bench(in_maps, warmup=3, iters=30):
    """Time repeated executions of the compiled 8-core kernel with inputs
    kept on device. Returns avg seconds per call (upper bound on HW time:
    includes dispatch)."""
    import time
    import jax
    from jax.sharding import Mesh, PartitionSpec
    from jax.experimental.shard_map import shard_map
    from concourse import bass2jax
    from concourse.bass2jax import _bass_exec_p, partition_id_tensor, install_neuronx_cc_hook

    global _NC_CACHE
    if _NC_CACHE is None:
        _NC_CACHE = build()
    nc = _NC_CACHE
    install_neuronx_cc_hook()
    n_cores = 8
    partition_name = nc.partition_id_tensor.name if nc.partition_id_tensor else None
    in_names, out_names, out_avals, zero_outs = [], [], [], []
    for alloc in nc.m.functions[0].allocations:
        if not isinstance(alloc, mybir.MemoryLocationSet):
            continue
        name = alloc.memorylocations[0].name
        if alloc.kind == "ExternalInput":
            if name != partition_name:
                in_names.append(name)
        elif alloc.kind == "ExternalOutput":
            out_names.append(name)
            shape = tuple(alloc.tensor_shape)
            dtype = mybir.dt.np(alloc.dtype)
            out_avals.append(jax.core.ShapedArray(shape, dtype))
            zero_outs.append(np.zeros(shape, dtype))
    n_params = len(in_names)
    n_outs = len(out_avals)
    all_names = in_names + out_names + ([partition_name] if partition_name else [])

    def _body(*args):
        operands = list(args)
        if partition_name is not None:
            operands.append(partition_id_tensor())
        outs = _bass_exec_p.bind(
            *operands, out_avals=tuple(out_avals), in_names=tuple(all_names),
            out_names=tuple(out_names), lowering_input_output_aliases=(),
            sim_require_finite=True, sim_require_nnan=True, nc=nc)
        return tuple(outs)

    devices = jax.devices()[:n_cores]
    mesh = Mesh(np.asarray(devices), ("core",))
    donate = tuple(range(n_params, n_params + n_outs))
    sharded = jax.jit(
        shard_map(_body, mesh=mesh, in_specs=(PartitionSpec("core"),) * (n_params + n_outs),
                  out_specs=(PartitionSpec("core"),) * n_outs, check_rep=False),
        donate_argnums=donate, keep_unused=True)
    concat_in = [np.concatenate([np.asarray(in_maps[c][nm]) for c in range(n_cores)], axis=0)
                 for nm in in_names]
    import jax.numpy as jnp
    sharding = jax.sharding.NamedSharding(mesh, PartitionSpec("core"))
    dev_in = [jax.device_put(a, sharding) for a in concat_in]

    def fresh_zeros():
        return [jax.device_put(np.zeros((n_cores * z.shape[0], *z.shape[1:]), z.dtype), sharding)
                for z in zero_outs]

    for _ in range(warmup):
        outs = sharded(*dev_in, *fresh_zeros())
        jax.block_until_ready(outs)
    zsets = [fresh_zeros() for _ in range(iters)]
    jax.block_until_ready(zsets)
    t0 = time.time()
    all_outs = []
    for i in range(iters):
        all_outs.append(sharded(*dev_in, *zsets[i]))
    jax.block_until_ready(all_outs)
    t1 = time.time()
    return (t1 - t0) / iters
